# revision 31
# baseline (speedup 1.0000x reference)
"""Trainium2 Bass kernel for nn_Block (moe_routing): transformer block =
LN1 + rotary/pos + 16-head causal attention + residual, then LN2 +
top-2-of-8-expert MoE FFN + residual.

Sharding over 8 NeuronCores:
  - tokens: core c owns tokens [b, 512j:512j+512] with b=c//4, j=c%4
    (attention is token-sharded; K/V computed over the full batch locally,
    causality enforced with a per-core additive mask input).
  - experts: core c owns expert c (expert-parallel MoE). h2 + gate logits
    are AllGathered, each core routes/gathers the tokens assigned to its
    expert, runs the FFN, scatters scaled outputs, and a bf16
    ReduceScatter sums expert contributions back to token owners.

All device activations are kept transposed ([D(part), tokens(free)]) so
every matmul contracts over the partition axis with no transposes except
(a) h2 rows for the AllGather/gather and (b) the final residual.
"""

import math
import os
import sys

import numpy as np

sys.path.insert(0, "/opt/trn_rl_repo")

import concourse.bass as bass  # noqa: E402
import concourse.tile as tile  # noqa: E402
from concourse import bacc, mybir  # noqa: E402
from concourse.alu_op_type import AluOpType  # noqa: E402
from concourse.masks import make_identity  # noqa: E402

AF = mybir.ActivationFunctionType
FP32 = mybir.dt.float32
BF16 = mybir.dt.bfloat16
I32 = mybir.dt.int32
P = 128
NCORE = 8
EPS = 1e-5


class Cfg:
    def __init__(self, T=2048, D=1024, H=16, F=4096, CAP=1280, MOE_CHUNK=640):
        self.B = 2
        self.T = T
        self.D = D
        self.H = H
        self.HD = D // H
        self.F = F
        self.E = 8
        self.CAP = CAP
        self.N = self.B * T            # total tokens
        self.TL = self.N // NCORE      # tokens per core
        self.DC = D // P               # D chunks
        self.KT = T // P               # key tiles
        self.TLT = self.TL // P        # local token tiles
        self.PAIRS = H // 2
        self.FT = F // P               # F tiles
        self.CI = CAP // P             # capacity tiles
        self.MOE_CHUNK = MOE_CHUNK     # slots per MoE token chunk
        self.MCN = CAP // MOE_CHUNK    # number of MoE chunks
        self.MCT = MOE_CHUNK // P      # 128-tiles per MoE chunk
        assert self.HD == 64 and H % 2 == 0 and self.E == 8
        assert T % 512 == 0 and D % P == 0 and F % P == 0
        assert CAP % MOE_CHUNK == 0 and MOE_CHUNK % P == 0
        assert self.TL % P == 0 and self.N % 16 == 0


def _nslices(n, step=512):
    return [(i, min(step, n - i)) for i in range(0, n, step)]


def build_nc(cfg: Cfg):
    """Build the SPMD Bass program (same program on all 8 cores)."""
    c = cfg
    nc = bacc.Bacc("TRN2", target_bir_lowering=False, debug=False,
                   num_devices=NCORE)
    RG = [list(range(NCORE))]

    # ---------------- I/O ----------------
    def din(name, shape, dt=FP32):
        return nc.dram_tensor(name, list(shape), dt, kind="ExternalInput").ap()

    xT = din("xT", (c.D, c.T))
    xqT = din("xqT", (c.D, c.TL))
    posT = din("posT", (c.D, c.T))
    posqT = din("posqT", (c.D, c.TL))
    wq = din("wq", (c.PAIRS, P, c.DC, P), BF16)   # pre-scaled, tiled
    wk = din("wk", (c.PAIRS, P, c.DC, P), BF16)
    wv = din("wv", (c.PAIRS, P, c.DC, P), BF16)
    wpT = din("wpT", (c.DC, P, c.DC, P), BF16)    # w_proj.T, tiled
    bproj = din("bproj", (c.D,))
    ln1g = din("ln1g", (c.D,))
    ln1b = din("ln1b", (c.D,))
    ln2g = din("ln2g", (c.D,))
    ln2b = din("ln2b", (c.D,))
    gwT = din("gwT", (c.D, c.E))
    w1 = din("w1", (c.FT, P, c.DC, P), BF16)
    b1 = din("b1", (c.F,))
    w2 = din("w2", (c.F, c.D), BF16)
    b2 = din("b2", (c.D,))
    maskT = din("maskT", (c.T, c.TL), BF16)   # multiplicative {0,1}
    onehot = din("onehot", (1, c.E))
    out = nc.dram_tensor("out", [c.TL, c.D], FP32, kind="ExternalOutput").ap()

    # ---------------- internal DRAM ----------------
    NPAD = c.N + 2 * P                      # pad rows for sentinel slots
    ROWW = c.D + P                          # h2 row + logits(bitcast) + pad
    ag_h2_in = nc.dram_tensor("ag_h2_in", [c.TL, ROWW], BF16).ap()
    h2_gt = nc.dram_tensor("h2_gt", [NPAD, c.D], BF16).ap()
    h2_full = nc.dram_tensor("h2_full", [NPAD, ROWW], BF16,
                             addr_space="Shared").ap()
    idtmp = nc.dram_tensor("idtmp", [c.N], FP32).ap()
    cwtmp = nc.dram_tensor("cwtmp", [c.N], FP32).ap()
    idx16_dram = nc.dram_tensor("idx16_dram", [16, c.CAP // 16],
                                mybir.dt.int16).ap()
    cwlin = nc.dram_tensor("cwlin", [c.CAP], FP32).ap()
    x2T_dram = nc.dram_tensor("x2T_dram", [c.D, c.TL], FP32).ap()
    moe_full = nc.dram_tensor("moe_full", [NPAD, c.D], BF16).ap()
    moe_slice = nc.dram_tensor("moe_slice", [c.TL, c.D], BF16).ap()

    NT16 = c.N // 16
    CAP16 = c.CAP // 16

    with tile.TileContext(nc) as tc:
        with tc.tile_pool(name="persist", bufs=1) as pp:
            # ---------------- constants ----------------
            ident = pp.tile([P, P], FP32)
            make_identity(nc, ident[:])
            ones = pp.tile([P, 1], FP32)
            nc.vector.memset(ones[:], 1.0)
            ones_bf = pp.tile([P, 1], BF16)
            nc.vector.memset(ones_bf[:], 1.0)

            def load_pcol(ap_dram, n):
                # [n*128] dram -> [128, n] sbuf (per-partition scalars)
                t = pp.tile([P, n], FP32, name=ap_dram.tensor.name + "_sb")
                nc.sync.dma_start(out=t[:], in_=ap_dram.rearrange(
                    "(a p) -> p a", p=P))
                return t

            bproj_sb = load_pcol(bproj, c.DC)
            ln1g_sb = load_pcol(ln1g, c.DC)
            ln1b_sb = load_pcol(ln1b, c.DC)
            ln2g_sb = load_pcol(ln2g, c.DC)
            ln2b_sb = load_pcol(ln2b, c.DC)
            b1_sb = load_pcol(b1, c.FT)

            gwT_sb = pp.tile([P, c.DC, c.E], FP32)
            nc.sync.dma_start(out=gwT_sb[:], in_=gwT.rearrange(
                "(a p) e -> p a e", p=P))
            onehot_sb = pp.tile([P, c.E], FP32)
            nc.sync.dma_start(out=onehot_sb[:], in_=bass.AP(
                tensor=onehot.tensor, offset=onehot.offset,
                ap=[[0, P]] + list(onehot.ap[1:])))

            x2r_all = pp.tile([P, c.TLT, c.D], FP32)  # x2 rows (for residual)

            # long-lived routing results
            cw_all = pp.tile([P, c.CI], FP32)        # combine weight per slot
            idx_w = pp.tile([P, CAP16], mybir.dt.int16)  # wrapped idx, x8

            # =========================================================
            # PHASE A: attention
            # =========================================================
            def ln_core(ctx_pool, ps_pool, xp, ntok, g_sb, b_sb, dt=FP32):
                """LN over partition axis of xp [128, DC, ntok] in place
                (center -> var -> scale), in dtype dt."""
                one_t = ones if dt == FP32 else ones_bf
                stat = ctx_pool.tile([1, ntok], FP32, tag="ln_stat", bufs=1)
                for ns, nn in _nslices(ntok):
                    ps_sum = ps_pool.tile([1, 512], FP32, tag="ln_ps1")
                    for dc in range(c.DC):
                        nc.tensor.matmul(ps_sum[:, :nn], one_t[:],
                                         xp[:, dc, ns:ns + nn],
                                         start=(dc == 0), stop=(dc == c.DC - 1))
                    nc.vector.tensor_copy(out=stat[:, ns:ns + nn],
                                          in_=ps_sum[:, :nn])
                nc.vector.tensor_scalar(out=stat[:], in0=stat[:],
                                        scalar1=1.0 / c.D, scalar2=None,
                                        op0=AluOpType.mult)
                statc = stat
                if dt != FP32:
                    statc = ctx_pool.tile([1, ntok], dt, tag="ln_statc",
                                          bufs=1)
                    nc.vector.tensor_copy(out=statc[:], in_=stat[:])
                # center xp in place (slice-wise broadcast of mu)
                for ns, nn in _nslices(ntok):
                    bc = ctx_pool.tile([P, 512], dt, tag="ln_bc")
                    nc.gpsimd.partition_broadcast(out_ap=bc[:, :nn],
                                                  in_ap=statc[:, ns:ns + nn])
                    for dc in range(c.DC):
                        nc.vector.tensor_tensor(out=xp[:, dc, ns:ns + nn],
                                                in0=xp[:, dc, ns:ns + nn],
                                                in1=bc[:, :nn],
                                                op=AluOpType.subtract)
                # variance of centered values
                stat2 = ctx_pool.tile([1, ntok], FP32, tag="ln_stat2", bufs=1)
                for ns, nn in _nslices(ntok):
                    ps_sq = ps_pool.tile([1, 512], FP32, tag="ln_ps2")
                    for dc in range(c.DC):
                        sq = ctx_pool.tile([P, 512], dt, tag="ln_sq")
                        nc.scalar.activation(out=sq[:, :nn],
                                             in_=xp[:, dc, ns:ns + nn],
                                             func=AF.Square)
                        nc.tensor.matmul(ps_sq[:, :nn], one_t[:], sq[:, :nn],
                                         start=(dc == 0), stop=(dc == c.DC - 1))
                    nc.vector.tensor_copy(out=stat2[:, ns:ns + nn],
                                          in_=ps_sq[:, :nn])
                nc.vector.tensor_scalar(out=stat2[:], in0=stat2[:],
                                        scalar1=1.0 / c.D, scalar2=EPS,
                                        op0=AluOpType.mult, op1=AluOpType.add)
                nc.scalar.activation(out=stat2[:], in_=stat2[:], func=AF.Sqrt)
                nc.vector.reciprocal(out=stat2[:], in_=stat2[:])
                stat2c = stat2
                if dt != FP32:
                    stat2c = ctx_pool.tile([1, ntok], dt, tag="ln_statc",
                                           bufs=1)
                    nc.vector.tensor_copy(out=stat2c[:], in_=stat2[:])
                for ns, nn in _nslices(ntok):
                    bc = ctx_pool.tile([P, 512], dt, tag="ln_bc")
                    nc.gpsimd.partition_broadcast(out_ap=bc[:, :nn],
                                                  in_ap=stat2c[:, ns:ns + nn])
                    for dc in range(c.DC):
                        t = ctx_pool.tile([P, 512], dt, tag="ln_t")
                        nc.vector.tensor_tensor(out=t[:, :nn],
                                                in0=xp[:, dc, ns:ns + nn],
                                                in1=bc[:, :nn],
                                                op=AluOpType.mult)
                        nc.vector.tensor_scalar(out=xp[:, dc, ns:ns + nn],
                                                in0=t[:, :nn],
                                                scalar1=g_sb[:, dc:dc + 1],
                                                scalar2=b_sb[:, dc:dc + 1],
                                                op0=AluOpType.mult,
                                                op1=AluOpType.add)

            def layernorm_T(ctx_pool, ps_pool, x_in, pos_in, ntok,
                            g_sb, b_sb, out_tile, tag, dt=FP32):
                """In: dram [D, ntok] (+pos). Out: out_tile [128,DC,ntok]
                layernormed, transposed layout."""
                xp = out_tile
                for dc in range(c.DC):
                    sl = slice(dc * P, (dc + 1) * P)
                    for ns, nn in _nslices(ntok):
                        xc = ctx_pool.tile([P, 512], FP32, tag="ln_ld", bufs=3)
                        rc = ctx_pool.tile([P, 512], FP32, tag="ln_ld2", bufs=3)
                        nc.sync.dma_start(out=xc[:, :nn],
                                          in_=x_in[sl, ns:ns + nn])
                        nc.sync.dma_start(out=rc[:, :nn],
                                          in_=pos_in[sl, ns:ns + nn])
                        nc.vector.tensor_tensor(out=xp[:, dc, ns:ns + nn],
                                                in0=xc[:, :nn],
                                                in1=rc[:, :nn],
                                                op=AluOpType.add)
                ln_core(ctx_pool, ps_pool, xp, ntok, g_sb, b_sb, dt=dt)

            with tc.tile_pool(name="attn", bufs=1) as ap_, \
                 tc.tile_pool(name="attn2", bufs=2) as ap2, \
                 tc.tile_pool(name="attn3", bufs=3) as ap3:

                hT = ap_.tile([P, c.DC, c.T], BF16)       # full-batch h (ln1)
                hqT = ap_.tile([P, c.DC, c.TL], BF16)     # my-block h (ln1)
                with tc.tile_pool(name="ln_ps", bufs=2, space="PSUM") as lnps:
                    layernorm_T(ap2, lnps, xT, posT, c.T,
                                ln1g_sb, ln1b_sb, hT, "lnf", dt=BF16)
                    layernorm_T(ap2, lnps, xqT, posqT, c.TL,
                                ln1g_sb, ln1b_sb, hqT, "lnq", dt=BF16)

                catT = ap_.tile([P, c.PAIRS, c.TL], BF16)

                pair_ps = tc.tile_pool(name="pair_ps", bufs=2, space="PSUM")
                pair_ps1 = tc.tile_pool(name="pair_ps1", bufs=1, space="PSUM")
                aps, aps1 = pair_ps.__enter__(), pair_ps1.__enter__()
                for p_ in range(c.PAIRS):
                    wq_t = ap2.tile([P, c.DC, P], BF16, tag="w_t", bufs=3)
                    wk_t = ap2.tile([P, c.DC, P], BF16, tag="w_t", bufs=3)
                    wv_t = ap2.tile([P, c.DC, P], BF16, tag="w_t", bufs=3)
                    nc.sync.dma_start(out=wq_t[:], in_=wq[p_, :, :, :])
                    nc.sync.dma_start(out=wk_t[:], in_=wk[p_, :, :, :])
                    nc.sync.dma_start(out=wv_t[:], in_=wv[p_, :, :, :])

                    # qT for this pair: [128(2 heads x 64), TL]
                    qT = ap2.tile([P, c.TL], BF16, tag="qT", bufs=2)
                    for ns, nn in _nslices(c.TL):
                        ps_q = aps1.tile([P, 512], FP32, tag="ps_q")
                        for dc in range(c.DC):
                            nc.tensor.matmul(ps_q[:, :nn], wq_t[:, dc, :],
                                             hqT[:, dc, ns:ns + nn],
                                             start=(dc == 0),
                                             stop=(dc == c.DC - 1))
                        nc.vector.tensor_copy(out=qT[:, ns:ns + nn],
                                              in_=ps_q[:, :nn])
                    # kT: [128, T]
                    kT = ap_.tile([P, c.T], BF16, tag="kT", bufs=2)
                    for ns, nn in _nslices(c.T):
                        ps_k = aps.tile([P, 512], FP32, tag="ps_k")
                        for dc in range(c.DC):
                            nc.tensor.matmul(ps_k[:, :nn], wk_t[:, dc, :],
                                             hT[:, dc, ns:ns + nn],
                                             start=(dc == 0),
                                             stop=(dc == c.DC - 1))
                        nc.vector.tensor_copy(out=kT[:, ns:ns + nn],
                                              in_=ps_k[:, :nn])
                    # v rows + ones col: v_aug [128, KT, 2, 66]
                    v_aug = ap_.tile([P, c.KT, 2, 66], BF16, tag="v_aug", bufs=2)
                    nc.vector.memset(v_aug[:, :, :, 64:65], 1.0)
                    for kt in range(c.KT):
                        ps_v = aps1.tile([P, P], FP32, tag="ps_v")
                        ksl = slice(kt * P, (kt + 1) * P)
                        for dc in range(c.DC):
                            nc.tensor.matmul(ps_v[:], hT[:, dc, ksl],
                                             wv_t[:, dc, :],
                                             start=(dc == 0),
                                             stop=(dc == c.DC - 1))
                        nc.vector.tensor_copy(
                            out=v_aug[:, kt, :, 0:64],
                            in_=ps_v[:].rearrange("p (h e) -> p h e", h=2))

                    # scores -> mask -> exp -> AV (accumulate over kt)
                    ps_av0 = aps1.tile([65, c.TL], FP32, tag="ps_av0")
                    ps_av1 = aps1.tile([65, c.TL], FP32, tag="ps_av1")
                    ps_avs = (ps_av0, ps_av1)
                    for kt in range(c.KT):
                        mt = ap3.tile([P, c.TL], BF16, tag="mt", bufs=4)
                        nc.sync.dma_start(out=mt[:],
                                          in_=maskT[kt * P:(kt + 1) * P, :])
                        for h2 in range(2):
                            hsl = slice(h2 * 64, (h2 + 1) * 64)
                            ps_s = aps.tile([P, c.TL], FP32, tag="ps_s")
                            nc.tensor.matmul(ps_s[:], kT[hsl, kt * P:(kt + 1) * P],
                                             qT[hsl, :], start=True, stop=True)
                            et = ap3.tile([P, c.TL], BF16, tag="et", bufs=4)
                            nc.scalar.activation(out=et[:], in_=ps_s[:],
                                                 func=AF.Exp)
                            nc.vector.tensor_tensor(out=et[:], in0=et[:],
                                                    in1=mt[:],
                                                    op=AluOpType.mult)
                            nc.tensor.matmul(ps_avs[h2][:],
                                             v_aug[:, kt, h2, 0:65], et[:],
                                             start=(kt == 0),
                                             stop=(kt == c.KT - 1))
                    # normalize; head 2p -> catT rows 0:64 direct,
                    # head 2p+1 -> rows 64:128 via DMA partition shift
                    for h2 in range(2):
                        rec = ap2.tile([1, c.TL], FP32, tag="rec", bufs=1)
                        nc.vector.reciprocal(out=rec[:],
                                             in_=ps_avs[h2][64:65, :])
                        rec_bc = ap2.tile([64, c.TL], FP32, tag="rec_bc", bufs=1)
                        nc.gpsimd.partition_broadcast(out_ap=rec_bc[:],
                                                      in_ap=rec[:])
                        if h2 == 0:
                            nc.vector.tensor_tensor(
                                out=catT[0:64, p_, :], in0=ps_avs[0][0:64, :],
                                in1=rec_bc[:], op=AluOpType.mult)
                        else:
                            shf = ap2.tile([64, c.TL], BF16, tag="shf", bufs=1)
                            nc.vector.tensor_tensor(
                                out=shf[:], in0=ps_avs[1][0:64, :],
                                in1=rec_bc[:], op=AluOpType.mult)
                            nc.sync.dma_start(out=catT[64:128, p_, :],
                                              in_=shf[:])

                pair_ps1.__exit__(None, None, None)
                pair_ps.__exit__(None, None, None)

                # proj: x2 = xq + catT @ wproj (+bproj), into h2T
                # (h2T holds x2 values first; LN2 runs in place after the
                # x2 copy is saved to DRAM for the final residual)
                proj_ps = tc.tile_pool(name="proj_ps", bufs=2, space="PSUM")
                aps = proj_ps.__enter__()
                h2T = ap_.tile([P, c.DC, c.TL], FP32)
                for dco in range(c.DC):
                    xq_res = ap2.tile([P, c.TL], FP32, tag="xq_res")
                    nc.sync.dma_start(out=xq_res[:],
                                      in_=xqT[dco * P:(dco + 1) * P, :])
                    wp_t = ap2.tile([P, c.DC, P], BF16, tag="wp_t", bufs=1)
                    nc.sync.dma_start(out=wp_t[:], in_=wpT[dco, :, :, :])
                    for ns, nn in _nslices(c.TL):
                        ps_p = aps.tile([P, 512], FP32, tag="ps_p")
                        for jc in range(c.PAIRS):
                            nc.tensor.matmul(ps_p[:, :nn], wp_t[:, jc, :],
                                             catT[:, jc, ns:ns + nn],
                                             start=(jc == 0),
                                             stop=(jc == c.PAIRS - 1))
                        t = ap2.tile([P, 512], FP32, tag="x2t", bufs=1)
                        nc.vector.tensor_scalar(
                            out=t[:, :nn], in0=ps_p[:, :nn],
                            scalar1=bproj_sb[:, dco:dco + 1], scalar2=None,
                            op0=AluOpType.add)
                        nc.vector.tensor_tensor(
                            out=h2T[:, dco, ns:ns + nn], in0=t[:, :nn],
                            in1=xq_res[:, ns:ns + nn], op=AluOpType.add)
                        nc.sync.dma_start(
                            out=x2T_dram[dco * P:(dco + 1) * P, ns:ns + nn],
                            in_=h2T[:, dco, ns:ns + nn])
                proj_ps.__exit__(None, None, None)
                tail_ps = tc.tile_pool(name="tail_ps", bufs=2, space="PSUM")
                aps = tail_ps.__enter__()

                # LN2 in place on h2T (x2 -> h2)
                ln_core(ap2, aps, h2T, c.TL, ln2g_sb, ln2b_sb)

                # gate logits for my block: [128, TLT, E]
                lg_loc = ap_.tile([P, c.TLT, c.E], FP32)
                for tt in range(c.TLT):
                    ps_l = aps.tile([P, c.E], FP32, tag="ps_l")
                    tsl = slice(tt * P, (tt + 1) * P)
                    for dc in range(c.DC):
                        nc.tensor.matmul(ps_l[:], h2T[:, dc, tsl],
                                         gwT_sb[:, dc, :],
                                         start=(dc == 0), stop=(dc == c.DC - 1))
                    nc.vector.tensor_copy(out=lg_loc[:, tt, :], in_=ps_l[:])
                nc.sync.dma_start(
                    out=ag_h2_in[:, c.D:c.D + 2 * c.E].rearrange(
                        "(tt p) e -> p tt e", p=P),
                    in_=lg_loc[:].bitcast(BF16))

                # h2 rows (token-major) for AllGather: PE transpose
                for tt in range(c.TLT):
                    for dc in range(c.DC):
                        ps_t = aps.tile([P, P], FP32, tag="ps_tr")
                        nc.tensor.transpose(
                            out=ps_t[:],
                            in_=h2T[:, dc, tt * P:(tt + 1) * P],
                            identity=ident[:])
                        h2r = ap2.tile([P, P], BF16, tag="h2r")
                        nc.vector.tensor_copy(out=h2r[:], in_=ps_t[:])
                        nc.sync.dma_start(
                            out=ag_h2_in[tt * P:(tt + 1) * P,
                                         dc * P:(dc + 1) * P],
                            in_=h2r[:])
                tail_ps.__exit__(None, None, None)

            # =========================================================
            # PHASE B: AllGather h2 + logits
            # =========================================================
            nc.gpsimd.collective_compute(
                "AllGather", AluOpType.bypass, replica_groups=RG,
                ins=[ag_h2_in[:]], outs=[h2_full[0:c.N, :]])

            # =========================================================
            # PHASE C: routing (top-2 of 8, combine weights, compaction)
            # =========================================================
            NTT = c.N // P
            with tc.tile_pool(name="route", bufs=1) as rp, \
                 tc.tile_pool(name="route2", bufs=2) as rp2, \
                 tc.tile_pool(name="route_ps", bufs=2, space="PSUM") as rps:
                # x2 rows for the final residual: fill the collective bubble
                # with the PE transposes (x2T_dram was final after proj)
                for tt in range(c.TLT):
                    for dc in range(c.DC):
                        x2c = rp2.tile([P, P], FP32, tag="x2c")
                        nc.sync.dma_start(
                            out=x2c[:],
                            in_=x2T_dram[dc * P:(dc + 1) * P,
                                         tt * P:(tt + 1) * P])
                        ps_t = rps.tile([P, P], FP32, tag="rps_tr")
                        nc.tensor.transpose(out=ps_t[:], in_=x2c[:],
                                            identity=ident[:])
                        nc.vector.tensor_copy(
                            out=x2r_all[:, tt, dc * P:(dc + 1) * P],
                            in_=ps_t[:])
                # compact h2 gather table (strip the logit columns)
                for gi in range(c.N // 512):
                    gtile = rp2.tile([P, 4, c.D], BF16, tag="gt")
                    nc.sync.dma_start(out=gtile[:], in_=h2_full[
                        gi * 512:(gi + 1) * 512, 0:c.D].rearrange(
                        "(a p) d -> p a d", p=P))
                    nc.sync.dma_start(out=h2_gt[
                        gi * 512:(gi + 1) * 512, :].rearrange(
                        "(a p) d -> p a d", p=P), in_=gtile[:])
                lg = rp.tile([P, NTT, c.E], FP32)
                nc.sync.dma_start(out=lg[:], in_=h2_full[
                    0:c.N, c.D:c.D + 2 * c.E].bitcast(FP32).rearrange(
                    "(a p) e -> p a e", p=P))
                m1 = rp.tile([P, NTT], FP32)
                nc.vector.tensor_reduce(out=m1[:], in_=lg[:],
                                        axis=mybir.AxisListType.X,
                                        op=AluOpType.max)
                # my logit = sum(lg * onehot)
                myl = rp.tile([P, NTT], FP32)
                tmp = rp.tile([P, NTT, c.E], FP32)
                for a in range(NTT):
                    nc.vector.tensor_tensor(out=tmp[:, a, :], in0=lg[:, a, :],
                                            in1=onehot_sb[:, :],
                                            op=AluOpType.mult)
                nc.vector.tensor_reduce(out=myl[:], in_=tmp[:],
                                        axis=mybir.AxisListType.X,
                                        op=AluOpType.add)
                # second max: mask out the argmax then reduce again
                eq1 = rp.tile([P, NTT, c.E], FP32)
                for a in range(NTT):
                    nc.vector.tensor_scalar(out=eq1[:, a, :], in0=lg[:, a, :],
                                            scalar1=m1[:, a:a + 1],
                                            scalar2=None,
                                            op0=AluOpType.is_equal)
                nc.vector.tensor_scalar(out=eq1[:], in0=eq1[:],
                                        scalar1=-1e30, scalar2=None,
                                        op0=AluOpType.mult)
                nc.vector.tensor_tensor(out=tmp[:], in0=lg[:], in1=eq1[:],
                                        op=AluOpType.add)
                m2 = rp.tile([P, NTT], FP32)
                nc.vector.tensor_reduce(out=m2[:], in_=tmp[:],
                                        axis=mybir.AxisListType.X,
                                        op=AluOpType.max)
                # w1 = sigmoid(m1-m2), w2 = 1-w1
                d12 = rp.tile([P, NTT], FP32)
                nc.vector.tensor_tensor(out=d12[:], in0=m1[:], in1=m2[:],
                                        op=AluOpType.subtract)
                w1_ = rp.tile([P, NTT], FP32)
                nc.scalar.activation(out=w1_[:], in_=d12[:], func=AF.Sigmoid)
                w2_ = rp.tile([P, NTT], FP32)
                nc.vector.tensor_scalar(out=w2_[:], in0=w1_[:], scalar1=-1.0,
                                        scalar2=1.0, op0=AluOpType.mult,
                                        op1=AluOpType.add)
                # flag = my_logit >= m2 ; eqtop = my_logit == m1
                flag = rp.tile([P, NTT], FP32)
                nc.vector.tensor_tensor(out=flag[:], in0=myl[:], in1=m2[:],
                                        op=AluOpType.is_ge)
                eqt = rp.tile([P, NTT], FP32)
                nc.vector.tensor_tensor(out=eqt[:], in0=myl[:], in1=m1[:],
                                        op=AluOpType.is_equal)
                # cw = flag * (w2 + eqt*(w1-w2))
                dw = rp.tile([P, NTT], FP32)
                nc.vector.tensor_tensor(out=dw[:], in0=w1_[:], in1=w2_[:],
                                        op=AluOpType.subtract)
                nc.vector.tensor_tensor(out=dw[:], in0=eqt[:], in1=dw[:],
                                        op=AluOpType.mult)
                nc.vector.tensor_tensor(out=dw[:], in0=dw[:], in1=w2_[:],
                                        op=AluOpType.add)
                cw = rp.tile([P, NTT], FP32)
                nc.vector.tensor_tensor(out=cw[:], in0=flag[:], in1=dw[:],
                                        op=AluOpType.mult)
                # idvals = flag*(tokid+1) - 1 ; cwvals = flag*(cw+1) - 1
                iota_i = rp.tile([P, NTT], I32)
                nc.gpsimd.iota(iota_i[:], pattern=[[P, NTT]], base=1,
                               channel_multiplier=1)
                iota_f = rp.tile([P, NTT], FP32)
                nc.vector.tensor_copy(out=iota_f[:], in_=iota_i[:])
                idv = rp.tile([P, NTT], FP32)
                nc.vector.tensor_tensor(out=idv[:], in0=flag[:], in1=iota_f[:],
                                        op=AluOpType.mult)
                nc.vector.tensor_scalar(out=idv[:], in0=idv[:], scalar1=-1.0,
                                        scalar2=None, op0=AluOpType.add)
                cwv = rp.tile([P, NTT], FP32)
                nc.vector.tensor_scalar(out=cwv[:], in0=cw[:], scalar1=1.0,
                                        scalar2=None, op0=AluOpType.add)
                nc.vector.tensor_tensor(out=cwv[:], in0=flag[:], in1=cwv[:],
                                        op=AluOpType.mult)
                nc.vector.tensor_scalar(out=cwv[:], in0=cwv[:], scalar1=-1.0,
                                        scalar2=None, op0=AluOpType.add)
                # to wrapped [16, N/16] layout via DRAM round-trip
                nc.sync.dma_start(out=bass.AP(
                    tensor=idtmp.tensor, offset=idtmp.offset,
                    ap=[[1, P], [P, NTT]]), in_=idv[:])
                nc.sync.dma_start(out=bass.AP(
                    tensor=cwtmp.tensor, offset=cwtmp.offset,
                    ap=[[1, P], [P, NTT]]), in_=cwv[:])
                sg_id = rp.tile([16, NT16 + CAP16], FP32)
                sg_cw = rp.tile([16, NT16 + CAP16], FP32)
                nc.vector.memset(sg_id[:, NT16:], float(c.N))  # sentinel id
                nc.vector.memset(sg_cw[:, NT16:], 0.0)         # sentinel cw
                nc.sync.dma_start(out=sg_id[:, 0:NT16], in_=bass.AP(
                    tensor=idtmp.tensor, offset=idtmp.offset,
                    ap=[[1, 16], [16, NT16]]))
                nc.sync.dma_start(out=sg_cw[:, 0:NT16], in_=bass.AP(
                    tensor=cwtmp.tensor, offset=cwtmp.offset,
                    ap=[[1, 16], [16, NT16]]))
                idsel = rp.tile([16, NT16 + CAP16], FP32)
                cwsel = rp.tile([16, NT16 + CAP16], FP32)
                nf1 = rp.tile([1, 1], mybir.dt.uint32)
                nf2 = rp.tile([1, 1], mybir.dt.uint32)
                nc.gpsimd.sparse_gather(out=idsel[:], in_=sg_id[:],
                                        num_found=nf1[:])
                nc.gpsimd.sparse_gather(out=cwsel[:], in_=sg_cw[:],
                                        num_found=nf2[:])
                # idx: wrapped fp32 -> int16, then replicate to the 8
                # gpsimd core groups via DRAM
                idsel16 = rp.tile([16, CAP16], mybir.dt.int16)
                nc.vector.tensor_copy(out=idsel16[:], in_=idsel[:, 0:CAP16])
                nc.sync.dma_start(out=idx16_dram[:, :], in_=idsel16[:])
                for g in range(8):
                    nc.sync.dma_start(out=idx_w[g * 16:(g + 1) * 16, :],
                                      in_=idx16_dram[:, :])
                # cw: wrapped -> slot-major [128, CI] via DRAM
                nc.sync.dma_start(out=bass.AP(
                    tensor=cwlin.tensor, offset=cwlin.offset,
                    ap=[[1, 16], [16, CAP16]]), in_=cwsel[:, 0:CAP16])
                nc.sync.dma_start(out=cw_all[:], in_=bass.AP(
                    tensor=cwlin.tensor, offset=cwlin.offset,
                    ap=[[1, P], [P, c.CI]]))
                # zero moe_full + h2_full pad rows via broadcast DMA
                zt = rp.tile([P, ROWW], BF16)
                nc.vector.memset(zt[:], 0.0)
                nblk = NPAD // P
                nc.sync.dma_start(
                    out=moe_full.rearrange("(a p) d -> p a d", p=P),
                    in_=bass.AP(tensor=zt[:].tensor, offset=zt[:].offset,
                                ap=[list(zt[:].ap[0]), [0, nblk],
                                    [1, c.D]]))
                nc.sync.dma_start(
                    out=h2_gt[c.N:NPAD, :].rearrange(
                        "(a p) d -> p a d", p=P),
                    in_=bass.AP(tensor=zt[:].tensor, offset=zt[:].offset,
                                ap=[list(zt[:].ap[0]), [0, 2],
                                    [1, c.D]]))

            # =========================================================
            # PHASE D: expert FFN over CAP slots in chunks (bf16)
            # =========================================================
            MC16 = c.MOE_CHUNK // 16
            with tc.tile_pool(name="moe1", bufs=1) as mp1, \
                 tc.tile_pool(name="moe2", bufs=2) as mp2, \
                 tc.tile_pool(name="moe_ps", bufs=2, space="PSUM") as mps:
                b2_sb = mp1.tile([P, c.D], FP32)
                nc.sync.dma_start(out=b2_sb[:], in_=bass.AP(
                    tensor=b2.tensor, offset=b2.offset,
                    ap=[[0, P]] + list(b2.ap)))
                # w2 resident; w1 streamed per F-tile
                w2_sb = mp1.tile([P, c.FT, c.D], BF16)
                nc.sync.dma_start(out=w2_sb[:], in_=w2.rearrange(
                    "(o p) d -> p o d", p=P))
                for mc in range(c.MCN):
                    csl = slice(mc * MC16, (mc + 1) * MC16)
                    hsel = mp2.tile([P, c.DC, c.MOE_CHUNK], BF16, tag="hselT")
                    nc.gpsimd.dma_gather(
                        out_ap=hsel[:], in_ap=h2_gt[:],
                        idxs_ap=idx_w[:, csl],
                        num_idxs=c.MOE_CHUNK, num_idxs_reg=c.MOE_CHUNK,
                        elem_size=c.D, transpose=True)
                    # FFN1: hidT[ft] = relu(w1[:,ft].T @ hsel + b1[ft])
                    hidT = mp1.tile([P, c.FT, c.MOE_CHUNK], BF16, tag="hidT")
                    for ft in range(c.FT):
                        w1t = mp2.tile([P, c.DC, P], BF16, tag="w1t", bufs=4)
                        nc.sync.dma_start(out=w1t[:], in_=w1[ft, :, :, :])
                        for ns, nn in _nslices(c.MOE_CHUNK):
                            ps_h = mps.tile([P, 512], FP32, tag="ps_h")
                            for dc in range(c.DC):
                                nc.tensor.matmul(ps_h[:, :nn],
                                                 w1t[:, dc, :],
                                                 hsel[:, dc, ns:ns + nn],
                                                 start=(dc == 0),
                                                 stop=(dc == c.DC - 1))
                            nc.scalar.activation(
                                out=hidT[:, ft, ns:ns + nn], in_=ps_h[:, :nn],
                                func=AF.Relu, bias=b1_sb[:, ft:ft + 1])
                    # FFN2 + bias + cw scale, then one scatter-add per chunk
                    orow = mp2.tile([P, c.MCT, c.D], BF16, tag="orow")
                    for mt in range(c.MCT):
                        slotcol = mc * c.MCT + mt
                        for ns, nn in _nslices(c.D):
                            ps_o = mps.tile([P, 512], FP32, tag="ps_o")
                            for fc in range(c.FT):
                                nc.tensor.matmul(
                                    ps_o[:, :nn],
                                    hidT[:, fc, mt * P:(mt + 1) * P],
                                    w2_sb[:, fc, ns:ns + nn],
                                    start=(fc == 0), stop=(fc == c.FT - 1))
                            t = mp2.tile([P, 512], FP32, tag="ot")
                            nc.vector.tensor_tensor(out=t[:, :nn],
                                                    in0=ps_o[:, :nn],
                                                    in1=b2_sb[:, ns:ns + nn],
                                                    op=AluOpType.add)
                            nc.vector.tensor_scalar(
                                out=orow[:, mt, ns:ns + nn], in0=t[:, :nn],
                                scalar1=cw_all[:, slotcol:slotcol + 1],
                                scalar2=None, op0=AluOpType.mult)
                    nc.gpsimd.dma_scatter_add(
                        out_ap=moe_full[:], in_ap=orow[:],
                        idxs_ap=idx_w[:, csl],
                        num_idxs=c.MOE_CHUNK, num_idxs_reg=c.MOE_CHUNK,
                        elem_size=c.D)

            # =========================================================
            # PHASE E: ReduceScatter + final residual
            # =========================================================
            nc.gpsimd.collective_compute(
                "ReduceScatter", AluOpType.add, replica_groups=RG,
                ins=[moe_full[0:c.N, :]], outs=[moe_slice[:]])

            with tc.tile_pool(name="fin", bufs=2) as fp:
                for tt in range(c.TLT):
                    ms = fp.tile([P, c.D], BF16, tag="ms")
                    nc.sync.dma_start(out=ms[:],
                                      in_=moe_slice[tt * P:(tt + 1) * P, :])
                    msf = fp.tile([P, c.D], FP32, tag="msf")
                    nc.vector.tensor_copy(out=msf[:], in_=ms[:])
                    orow = fp.tile([P, c.D], FP32, tag="fout")
                    nc.vector.tensor_tensor(out=orow[:], in0=x2r_all[:, tt, :],
                                            in1=msf[:], op=AluOpType.add)
                    nc.sync.dma_start(out=out[tt * P:(tt + 1) * P, :],
                                      in_=orow[:])

    nc.compile()
    return nc


# =====================================================================
# Host side
# =====================================================================

def _rot_table(T, D):
    freqs = (np.arange(0, D, 2, dtype=np.float64) / D)
    t = np.arange(T, dtype=np.float64)
    ang = 2.0 * math.pi * t[:, None] * freqs[None, :]
    rot = np.stack([np.sin(ang), np.cos(ang)], axis=-1).reshape(T, D)
    return rot.astype(np.float32)


def _wtile(w, P_, nI, nO):
    # [nI*128, nO*128] -> [nO, 128, nI, 128]: tile (o) is a contiguous
    # [128p, nI, 128m] block (partition-major rows for single-descriptor DMA)
    return np.ascontiguousarray(
        w.reshape(nI, P_, nO, P_).transpose(2, 1, 0, 3))


def make_in_maps(cfg, x, pos_emb, wq, wk, wv, w_proj, b_proj, ln1_g, ln1_b,
                 ln2_g, ln2_b, gate_w, e_w1, e_b1, e_w2, e_b2):
    import ml_dtypes
    bf16 = ml_dtypes.bfloat16
    c = cfg
    f32 = np.float32
    x = np.asarray(x, f32)
    rot = _rot_table(c.T, c.D)
    pemb = np.asarray(pos_emb, f32)[:c.T]
    scale = c.D ** (-0.5)
    wq_cat = (np.asarray(wq, f32).transpose(1, 0, 2).reshape(c.D, c.D)
              * scale).copy()
    wk_cat = np.asarray(wk, f32).transpose(1, 0, 2).reshape(c.D, c.D).copy()
    wv_cat = np.asarray(wv, f32).transpose(1, 0, 2).reshape(c.D, c.D).copy()
    wpT = np.asarray(w_proj, f32).T.copy()
    gwT = np.asarray(gate_w, f32).T.copy()
    posT = np.ascontiguousarray((rot + pemb).T.astype(f32))
    wq_r = _wtile(wq_cat, P, c.DC, c.PAIRS).astype(bf16)
    wk_r = _wtile(wk_cat, P, c.DC, c.PAIRS).astype(bf16)
    wv_r = _wtile(wv_cat, P, c.DC, c.PAIRS).astype(bf16)
    wp_r = _wtile(wpT, P, c.DC, c.DC).astype(bf16)

    in_maps = []
    for core in range(NCORE):
        b, j = core // (NCORE // c.B), core % (NCORE // c.B)
        t0 = j * c.TL
        tok = np.arange(t0, t0 + c.TL)
        keys = np.arange(c.T)
        maskT = np.where(keys[:, None] <= tok[None, :], 1.0, 0.0).astype(bf16)
        onehot = np.zeros((1, c.E), f32)
        onehot[0, core % c.E] = 1.0
        m = {
            "xT": x[b].T.copy(),
            "xqT": x[b, t0:t0 + c.TL].T.copy(),
            "posT": posT,
            "posqT": np.ascontiguousarray(posT[:, t0:t0 + c.TL]),
            "wq": wq_r, "wk": wk_r, "wv": wv_r, "wpT": wp_r,
            "bproj": np.asarray(b_proj, f32),
            "ln1g": np.asarray(ln1_g, f32), "ln1b": np.asarray(ln1_b, f32),
            "ln2g": np.asarray(ln2_g, f32), "ln2b": np.asarray(ln2_b, f32),
            "gwT": gwT,
            "w1": _wtile(np.asarray(e_w1, f32)[core % c.E], P, c.DC,
                         c.FT).astype(bf16),
            "b1": np.asarray(e_b1, f32)[core % c.E].copy(),
            "w2": np.ascontiguousarray(
                np.asarray(e_w2, f32)[core % c.E]).astype(bf16),
            "b2": np.asarray(e_b2, f32)[core % c.E].copy(),
            "maskT": maskT,
            "onehot": onehot,
        }
        in_maps.append(m)
    return in_maps


_CACHE = {}
LAST_RESULTS = None


def _ensure_ntff_hook():
    """Inject antenv.axon_hooks (missing from this image) and install the
    ctypes NTFF profile hook against libaxon_pjrt.so so that
    run_bass_kernel_spmd(trace=True) can capture device profiles."""
    import contextlib
    import ctypes
    import types

    try:
        from antenv.axon_hooks import get_axon_ntff_profile_hook  # noqa: F401
        return True
    except ImportError:
        pass
    so_path = "/opt/axon/libaxon_pjrt.so"
    if not os.path.exists(so_path):
        return False
    lib = ctypes.CDLL(so_path)
    if not hasattr(lib, "axon_start_nrt_profile"):
        return False
    lib.axon_start_nrt_profile.argtypes = [ctypes.POINTER(ctypes.c_int64),
                                           ctypes.c_size_t]
    lib.axon_start_nrt_profile.restype = ctypes.c_int64
    lib.axon_stop_nrt_profile.argtypes = [ctypes.c_char_p]
    lib.axon_stop_nrt_profile.restype = ctypes.c_int64

    @contextlib.contextmanager
    def _hook(output_dir, device_ids):
        import jax
        jax.devices()
        if device_ids:
            ids = (ctypes.c_int64 * len(device_ids))(*device_ids)
            rc = lib.axon_start_nrt_profile(ids, len(device_ids))
        else:
            rc = lib.axon_start_nrt_profile(None, 0)
        if rc != 0:
            raise RuntimeError(f"axon_start_nrt_profile rc={rc}")
        try:
            yield
        finally:
            n = lib.axon_stop_nrt_profile(str(output_dir).encode())
            print(f"ntff profile: {n} file(s) -> {output_dir}",
                  file=sys.stderr)

    mod = types.ModuleType("antenv.axon_hooks")
    state = {"h": _hook}
    mod.set_axon_ntff_profile_hook = lambda h: state.__setitem__("h", h)
    mod.get_axon_ntff_profile_hook = lambda: state["h"]
    sys.modules["antenv.axon_hooks"] = mod
    import antenv
    antenv.axon_hooks = mod
    # avoid remote artifact upload in this container
    from concourse import bass_utils as _bu
    _bu.upload_artifacts = lambda tmpdir: tmpdir
    return True


def kernel(**inputs):
    """Full inputs in (as reference.setup_inputs), full output out."""
    cfg = Cfg()
    key = "full"
    if key not in _CACHE:
        _CACHE[key] = build_nc(cfg)
    nc = _CACHE[key]
    in_maps = make_in_maps(cfg, **{k: np.asarray(v) for k, v in inputs.items()})
    trace = bool(os.environ.get("KB_TRACE"))
    if trace:
        trace = _ensure_ntff_hook()
    from concourse.bass_utils import run_bass_kernel_spmd
    global LAST_RESULTS
    res = run_bass_kernel_spmd(nc, in_maps, list(range(NCORE)), trace=trace)
    LAST_RESULTS = res
    outs = [res.results[i]["out"] for i in range(NCORE)]
    c = cfg
    out = np.zeros((c.B, c.T, c.D), np.float32)
    for core in range(NCORE):
        b, j = core // (NCORE // c.B), core % (NCORE // c.B)
        out[b, j * c.TL:(j + 1) * c.TL] = outs[core]
    return out
